# revision 1
# baseline (speedup 1.0000x reference)
"""Power-STFT kernel for Trainium2 (8 NeuronCores, data-parallel over batch).

Computes, for x [32, 320000] and scalar lambd:
    x <- x - mean(x, axis=1)
    power-STFT (n_fft=1024, hop=320, periodic Hann, center reflect pad)
    out = log1p(lambd * power)   -> [32, 513, 1001] fp32

Strategy per core (4 batch samples per core), v2 — folded DFT in fp16:
  - Window/trig symmetry: win(1024-n) = win(n), cos sym / sin antisym about
    n=512, so the windowed DFT reduces to a length-512 contraction over
      u_t[n] = y_t[n] + y_t[1024-n],  v_t[n] = y_t[n] - y_t[1024-n]
    (n = 1..512; u[512] = 2*y[512] absorbed with half weight; n=0 has
    win(0) = 0). This HALVES the tensor-engine work vs the direct 8-chunk
    form: 4 contraction chunks of 128 per trig.
  - All matmul inputs are fp16 (full PE rate, 1 row/cycle). Host ships four
    slab views of the reflect-padded signal so every fold operand is
    partition-aligned: xa/xb (forward, offsets +1/+65 so contraction slot
    (c,p) = sample 128c+p+1) and xrevA/xrevB (reversed: xp[321023-i] and
    xp[320959-i]) for even/odd frames. u/v are built by 4 DVE adds/subs per
    sample (fp16 2x mode) into a [128, v*8+par*4+c] layout whose matmul rhs
    slices are [128, stride-8 x 501].
  - Nyquist bin (512) rides data-stationary matmuls: lhsT = u frames chunk
    (M = 128 frames), rhs = folded (-1)^n window column (F=1) accumulated
    over 4 chunks into a [128 frames, 8 fgroup] PSUM tile; squared, PE-
    transposed via identity matmul, Ln'd, and DMA'd as output row 512.
  - Mean removal: periodic-Hann DFT of a constant is exactly [512, -256]
    at bins 0/1 (real), zero elsewhere. So demeaning == biasing bins 0/1
    of the cos part: bias vec = mu * [-512, 256, 0...] * sqrt(lambd),
    applied as the per-partition bias of the kb=0 cos ACT Square. mu comes
    from a GPSIMD column-reduce of xa + a 3-matmul partition-sum with
    edge fixups.
  - Epilogue per (kb): cos^2 on ACT (Square, PSUM strided view skipping
    bank-pad junk), sin^2 on DVE (scalar_tensor_tensor (ps*1)*ps), power
    add split DVE/GPSIMD, log1p via ACT Ln(power + 1) writing the t-
    interleaved fp32 out tile; one 2MB DMA per sample for bins 0..511.
  - sqrt(lambd) folded into the DFT matrices so power is pre-scaled.
"""

import sys

sys.path.insert(0, "/opt/trn_rl_repo")

import numpy as np

import concourse.bacc as bacc
import concourse.bass as bass
import concourse.mybir as mybir
import concourse.tile as tile
from concourse.ap import AP
from contextlib import ExitStack

N_FFT = 1024
HOP = 320
L = 320000
PAD = N_FFT // 2  # 512
LP = L + 2 * PAD  # 321024
B = 32
NCORES = 8
SPC = B // NCORES  # 4 samples per core
T = 1 + L // HOP  # 1001 frames
NV = 501  # even-frame count; odd frames use 500 + 1 junk col
QS = 2506  # slab columns
NBINS = 513

_f32 = mybir.dt.float32
_f16 = mybir.dt.float16


def _ap3(t, col_off, s1, n1, s2, n2):
    """[128, n1, n2] AP on tile t with free strides (s1, s2) from col_off."""
    base = t[:, 0:1]
    return AP(base.tensor, base.offset + col_off,
              [list(base.ap[0]), [s1, n1], [s2, n2]])


def _build_module():
    nc = bacc.Bacc(None, target_bir_lowering=False, debug=False)

    x4_d = nc.dram_tensor("x4", [SPC, 4, 128, QS], _f16, kind="ExternalInput")
    wc_d = nc.dram_tensor("wc", [128, 4, 512], _f16, kind="ExternalInput")
    ws_d = nc.dram_tensor("ws", [128, 4, 512], _f16, kind="ExternalInput")
    wny_d = nc.dram_tensor("wny", [128, 4], _f16, kind="ExternalInput")
    tmpl_d = nc.dram_tensor("tmpl", [1, 128], _f32, kind="ExternalInput")
    ident_d = nc.dram_tensor("ident", [128, 128], _f16, kind="ExternalInput")
    o_d = nc.dram_tensor("o", [SPC, NBINS, T], _f32, kind="ExternalOutput")

    with tile.TileContext(nc) as tc:
        with ExitStack() as ctx:
            consts = ctx.enter_context(tc.tile_pool(name="consts", bufs=1))
            slabs = ctx.enter_context(tc.tile_pool(name="slabs", bufs=2))
            uvs = ctx.enter_context(tc.tile_pool(name="uvs", bufs=2))
            stats = ctx.enter_context(tc.tile_pool(name="stats", bufs=2))
            tmps = ctx.enter_context(tc.tile_pool(name="tmps", bufs=3))
            outs = ctx.enter_context(tc.tile_pool(name="outs", bufs=2))
            pmain = ctx.enter_context(tc.tile_pool(name="pmain", bufs=1, space="PSUM"))
            psmall = ctx.enter_context(tc.tile_pool(name="psmall", bufs=1, space="PSUM"))

            wc_sb = consts.tile([128, 4, 512], _f16)
            nc.sync.dma_start(out=wc_sb, in_=wc_d[:, :, :])
            ws_sb = consts.tile([128, 4, 512], _f16)
            nc.sync.dma_start(out=ws_sb, in_=ws_d[:, :, :])
            wny_sb = consts.tile([128, 4], _f16)
            nc.sync.dma_start(out=wny_sb, in_=wny_d[:, :])
            tmplP = consts.tile([1, 128], _f32)
            nc.sync.dma_start(out=tmplP, in_=tmpl_d[:, :])
            ident_sb = consts.tile([128, 128], _f16)
            nc.sync.dma_start(out=ident_sb, in_=ident_d[:, :])

            for s in range(SPC):
                xa = slabs.tile([128, QS], _f16, tag="xa", name=f"xa{s}")
                nc.sync.dma_start(out=xa, in_=x4_d[s, 0, :, :])
                xb = slabs.tile([128, QS], _f16, tag="xb", name=f"xb{s}")
                nc.sync.dma_start(out=xb, in_=x4_d[s, 1, :, :])
                xrevA = slabs.tile([128, QS], _f16, tag="xrA", name=f"xrA{s}")
                nc.sync.dma_start(out=xrevA, in_=x4_d[s, 2, :, :])
                xrevB = slabs.tile([128, QS], _f16, tag="xrB", name=f"xrB{s}")
                nc.sync.dma_start(out=xrevB, in_=x4_d[s, 3, :, :])

                # fold: u/v [128, 8v+4par+c], even par=0 from xa/xrevA,
                # odd par=1 from xb/xrevB (odd v=500 col is finite junk);
                # rev slabs are host-gathered in forward column order so all
                # strides are positive
                u = uvs.tile([128, 8 * NV], _f16, tag="u", name=f"u{s}")
                v = uvs.tile([128, 8 * NV], _f16, tag="v", name=f"v{s}")
                for par, fwd, rev, off_f in ((0, xa, xrevA, 0), (1, xb, xrevB, 2)):
                    in0 = _ap3(fwd, off_f, 5, NV, 1, 4)
                    in1 = _ap3(rev, 0, 5, NV, 1, 4)
                    uo = _ap3(u, 4 * par, 8, NV, 1, 4)
                    vo = _ap3(v, 4 * par, 8, NV, 1, 4)
                    nc.vector.tensor_add(out=uo, in0=in0, in1=in1)
                    nc.vector.tensor_sub(out=vo, in0=in0, in1=in1)

                # mean chain: sum(xa[:, 4:2504]) == sum(x) up to two edge
                # samples (~1e-5 relative on mu — far below tolerance);
                # bias = tmpl * S (tmpl carries /L and sqrt(lambd) scaling)
                sS = stats.tile([1, 1], _f32, tag="sS", name=f"sS{s}")
                nc.gpsimd.reduce_sum(out=sS, in_=xa[:, 4:2504],
                                     axis=mybir.AxisListType.XYZWC)
                # one shared PSUM bank per sample: pny cols 0:8, pT 128:256,
                # pbias col 256
                misc = psmall.tile([128, 512], _f32, tag="misc", bufs=2,
                                   name=f"misc{s}")
                nc.tensor.matmul(misc[:, 256:257], lhsT=tmplP[:, :],
                                 rhs=sS[:, :], start=True, stop=True)
                bias_sb = stats.tile([128, 1], _f32, tag="bias", name=f"bias{s}")
                nc.scalar.activation(out=bias_sb, in_=misc[:, 256:257],
                                     func=mybir.ActivationFunctionType.Copy)

                # Nyquist bin: data-stationary chains, frames on out partitions
                pny = misc[:, 0:8]
                nc.vector.memset(misc[:, 7:8], 0.0)  # fg7 pads partitions 106+
                ubase = u[:, 0:1]
                for fg in range(8):
                    nv = 64 if fg < 7 else 53
                    m = 2 * nv
                    for c in range(4):
                        lhsT = AP(ubase.tensor, ubase.offset + 8 * 64 * fg + c,
                                  [list(ubase.ap[0]), [8, nv], [4, 2]])
                        nc.tensor.matmul(pny[0:m, fg:fg + 1], lhsT=lhsT,
                                         rhs=wny_sb[:, c:c + 1],
                                         start=(c == 0), stop=(c == 3))
                syT = stats.tile([128, 8], _f16, tag="syT", name=f"syT{s}")
                nc.scalar.activation(out=syT, in_=pny,
                                     func=mybir.ActivationFunctionType.Square)
                pT = misc[0:8, 128:256]
                nc.tensor.matmul(pT, lhsT=syT[:, :], rhs=ident_sb[:, :],
                                 start=True, stop=True)
                nyrow = stats.tile([8, 128], _f32, tag="nyrow", name=f"ny{s}")
                nc.scalar.activation(out=nyrow, in_=pT,
                                     func=mybir.ActivationFunctionType.Ln, bias=1.0)
                nc.sync.dma_start(
                    out=o_d[s, 512:513, 0:896].rearrange("one (fg j) -> (one fg) j", fg=7),
                    in_=nyrow[0:7, :],
                )
                nc.sync.dma_start(out=o_d[s, 512:513, 896:1001], in_=nyrow[7:8, 0:105])

                osb = outs.tile([128, 4 * 1002], _f32, tag="osb", name=f"osb{s}")
                for kb in range(4):
                    pc = pmain.tile([128, 1024], _f32, tag="pc", bufs=2, name=f"pc{s}_{kb}")
                    ps_ = pmain.tile([128, 1024], _f32, tag="ps", bufs=1, name=f"ps{s}_{kb}")
                    for par in range(2):
                        for c in range(4):
                            rhs = _ap3(u, 4 * par + c, 8, NV, 1, 1).squeeze(2)
                            nc.tensor.matmul(
                                pc[:, 512 * par:512 * par + NV],
                                lhsT=wc_sb[:, c, 128 * kb:128 * kb + 128],
                                rhs=rhs, start=(c == 0), stop=(c == 3))
                    for par in range(2):
                        for c in range(4):
                            rhs = _ap3(v, 4 * par + c, 8, NV, 1, 1).squeeze(2)
                            nc.tensor.matmul(
                                ps_[:, 512 * par:512 * par + NV],
                                lhsT=ws_sb[:, c, 128 * kb:128 * kb + 128],
                                rhs=rhs, start=(c == 0), stop=(c == 3))
                    # strided PSUM views skip the bank-pad junk cols
                    pcv = _ap3(pc, 0, 512, 2, 1, NV)
                    psv = _ap3(ps_, 0, 512, 2, 1, NV)
                    t1 = tmps.tile([128, 2 * NV], _f16, tag="t1", name=f"t1{s}_{kb}")
                    t1v = _ap3(t1, 0, NV, 2, 1, NV)
                    nc.scalar.activation(
                        out=t1v, in_=pcv, func=mybir.ActivationFunctionType.Square,
                        bias=(bias_sb[:, 0:1] if kb == 0 else 0.0))
                    t2 = tmps.tile([128, 2 * NV], _f16, tag="t2", name=f"t2{s}_{kb}")
                    t2v = _ap3(t2, 0, NV, 2, 1, NV)
                    if kb in (0, 2):  # ACT square (DVE can't dual-read PSUM)
                        nc.scalar.activation(
                            out=t2v, in_=psv,
                            func=mybir.ActivationFunctionType.Square)
                    else:  # DVE: copy PSUM->fp16, then 2x-mode self-mult
                        cp = tmps.tile([128, 2 * NV], _f16, tag="cp",
                                       name=f"cp{s}_{kb}")
                        cpv = _ap3(cp, 0, NV, 2, 1, NV)
                        nc.vector.tensor_copy(out=cpv, in_=psv)
                        nc.vector.tensor_mul(out=t2[:, :], in0=cp[:, :],
                                             in1=cp[:, :])
                    pw = tmps.tile([128, 2 * NV], _f16, tag="pw", name=f"pw{s}_{kb}")
                    eng = nc.vector if kb < 2 else nc.gpsimd
                    eng.tensor_add(out=pw[:, :], in0=t1[:, :], in1=t2[:, :])
                    # ln1p -> t-interleaved f32 out block
                    obase = osb[:, 0:1]
                    oap = AP(obase.tensor, obase.offset + 1002 * kb,
                             [list(obase.ap[0]), [1, 2], [2, NV]])
                    pwv = _ap3(pw, 0, NV, 2, 1, NV)
                    nc.scalar.activation(out=oap, in_=pwv,
                                         func=mybir.ActivationFunctionType.Ln, bias=1.0)
                nc.sync.dma_start(
                    out=o_d[s, 0:512, :].rearrange("(kb p) t -> p kb t", kb=4),
                    in_=_ap3(osb, 0, 1002, 4, 1, T),
                )

    nc.compile()
    return nc


def _host_prepare(x, lambd):
    """Build per-core slab inputs + folded DFT matrices (fp16)."""
    x = np.ascontiguousarray(x, dtype=np.float32)
    lam = float(np.asarray(lambd, dtype=np.float32))
    sq = np.sqrt(abs(lam)) if lam != 0 else 1.0

    xp = np.concatenate(
        [x[:, PAD:0:-1], x, x[:, L - 2: L - 2 - PAD: -1]], axis=1
    )  # [B, LP]
    nq = 128 * QS  # 320768 <= LP

    def slab(src, off):
        return np.ascontiguousarray(
            src[:, off:off + nq].reshape(B, QS, 128).transpose(0, 2, 1)
        ).astype(np.float16)

    xa = slab(xp, 1)   # xp[128q+p+1]
    xb = slab(xp, 65)  # xp[128q+p+65]

    # reversed-operand slabs, gathered in forward column order:
    # xrev*[p, 5v+c] = xp[base + 640v - 128c - p]
    vv = np.arange(NV)
    cc = np.arange(4)
    pp = np.arange(128)
    idx = (640 * vv[:, None, None] - 128 * cc[None, :, None]
           - pp[None, None, :])  # [NV, 4, 128]
    cols = (5 * vv[:, None] + cc[None, :]).ravel()  # 2004 used columns

    def revslab(base):
        iz = np.clip(base + idx, 0, LP - 1)
        vals = xp[:, iz]  # [B, NV, 4, 128]
        out = np.zeros((B, 128, QS), dtype=np.float16)
        out[:, :, cols] = vals.transpose(0, 3, 1, 2).reshape(B, 128, 4 * NV)
        return out

    xrevA = revslab(1023)  # xp[640v + 1023 - 128c - p]
    xrevB = revslab(1343)  # xp[640v + 1343 - 128c - p]
    x4 = np.ascontiguousarray(np.stack([xa, xb, xrevA, xrevB], axis=1))

    n = np.arange(1, 513, dtype=np.float64)  # contraction slots 1..512
    win = 0.5 * (1.0 - np.cos(2.0 * np.pi * n / N_FFT))
    k = np.arange(512, dtype=np.float64)
    ang = 2.0 * np.pi * np.outer(n, k) / N_FFT
    wc64 = sq * win[:, None] * np.cos(ang)
    ws64 = sq * win[:, None] * np.sin(ang)
    wc64[511, :] *= 0.5  # u[512] = 2*y[512]
    ws64[511, :] = 0.0
    wny64 = sq * win * np.cos(np.pi * n)
    wny64[511] = 0.5 * sq

    def to_pck(w):  # [512, nk] -> [128, 4, nk], slot n=128c+p+1
        return np.ascontiguousarray(
            w.reshape(4, 128, -1).transpose(1, 0, 2)).astype(np.float16)

    wc = to_pck(wc64)
    ws = to_pck(ws64)
    wny = np.ascontiguousarray(
        wny64.reshape(4, 128).transpose(1, 0)).astype(np.float16)
    tmpl = np.zeros((1, 128), dtype=np.float32)
    tmpl[0, 0] = -512.0 * sq / L
    tmpl[0, 1] = 256.0 * sq / L
    ident = np.eye(128, dtype=np.float16)
    return x4, wc, ws, wny, tmpl, ident


def _in_maps(x4, wc, ws, wny, tmpl, ident):
    maps = []
    for c in range(NCORES):
        sl = slice(c * SPC, (c + 1) * SPC)
        maps.append({
            "x4": np.ascontiguousarray(x4[sl]),
            "wc": wc, "ws": ws, "wny": wny, "tmpl": tmpl, "ident": ident,
        })
    return maps


def kernel(x, lambd):
    from concourse.bass_utils import run_bass_kernel_spmd

    prep = _host_prepare(x, lambd)
    nc = _build_module()
    res = run_bass_kernel_spmd(nc, _in_maps(*prep), core_ids=list(range(NCORES)))
    out = np.concatenate([res.results[c]["o"] for c in range(NCORES)], axis=0)
    return out.astype(np.float32)


if __name__ == "__main__":
    rng = np.random.default_rng(0)
    x = rng.standard_normal((B, L), dtype=np.float32)
    out = kernel(x, np.float32(5.0))
    print(out.shape, out.dtype, out[0, :3, :3])



# revision 5
# speedup vs baseline: 1.0319x; 1.0319x over previous
"""Power-STFT kernel for Trainium2 (8 NeuronCores, data-parallel over batch).

Computes, for x [32, 320000] and scalar lambd:
    x <- x - mean(x, axis=1)
    power-STFT (n_fft=1024, hop=320, periodic Hann, center reflect pad)
    out = log1p(lambd * power)   -> [32, 513, 1001] fp32

Strategy per core (4 batch samples per core), v2 — folded DFT in fp16:
  - Window/trig symmetry: win(1024-n) = win(n), cos sym / sin antisym about
    n=512, so the windowed DFT reduces to a length-512 contraction over
      u_t[n] = y_t[n] + y_t[1024-n],  v_t[n] = y_t[n] - y_t[1024-n]
    (n = 1..512; u[512] = 2*y[512] absorbed with half weight; n=0 has
    win(0) = 0). This HALVES the tensor-engine work vs the direct 8-chunk
    form: 4 contraction chunks of 128 per trig.
  - All matmul inputs are fp16 (full PE rate, 1 row/cycle). Host ships four
    slab views of the reflect-padded signal so every fold operand is
    partition-aligned: xa/xb (forward, offsets +1/+65 so contraction slot
    (c,p) = sample 128c+p+1) and xrevA/xrevB (reversed: xp[321023-i] and
    xp[320959-i]) for even/odd frames. u/v are built by 4 DVE adds/subs per
    sample (fp16 2x mode) into a [128, v*8+par*4+c] layout whose matmul rhs
    slices are [128, stride-8 x 501].
  - Nyquist bin (512) rides data-stationary matmuls: lhsT = u frames chunk
    (M = 128 frames), rhs = folded (-1)^n window column (F=1) accumulated
    over 4 chunks into a [128 frames, 8 fgroup] PSUM tile; squared, PE-
    transposed via identity matmul, Ln'd, and DMA'd as output row 512.
  - Mean removal: periodic-Hann DFT of a constant is exactly [512, -256]
    at bins 0/1 (real), zero elsewhere. So demeaning == biasing bins 0/1
    of the cos part: bias vec = mu * [-512, 256, 0...] * sqrt(lambd),
    applied as the per-partition bias of the kb=0 cos ACT Square. mu comes
    from a GPSIMD column-reduce of xa + a 3-matmul partition-sum with
    edge fixups.
  - Epilogue per (kb): cos^2 on ACT (Square, PSUM strided view skipping
    bank-pad junk), sin^2 on DVE (scalar_tensor_tensor (ps*1)*ps), power
    add split DVE/GPSIMD, log1p via ACT Ln(power + 1) writing the t-
    interleaved fp32 out tile; one 2MB DMA per sample for bins 0..511.
  - sqrt(lambd) folded into the DFT matrices so power is pre-scaled.
"""

import sys

sys.path.insert(0, "/opt/trn_rl_repo")

import numpy as np

import concourse.bacc as bacc
import concourse.bass as bass
import concourse.mybir as mybir
import concourse.tile as tile
from concourse.ap import AP
from contextlib import ExitStack

N_FFT = 1024
HOP = 320
L = 320000
PAD = N_FFT // 2  # 512
LP = L + 2 * PAD  # 321024
B = 32
NCORES = 8
SPC = B // NCORES  # 4 samples per core
T = 1 + L // HOP  # 1001 frames
NV = 501  # even-frame count; odd frames use 500 + 1 junk col
QS = 2506  # slab columns
NBINS = 513

_f32 = mybir.dt.float32
_f16 = mybir.dt.float16


def _ap3(t, col_off, s1, n1, s2, n2):
    """[128, n1, n2] AP on tile t with free strides (s1, s2) from col_off."""
    base = t[:, 0:1]
    return AP(base.tensor, base.offset + col_off,
              [list(base.ap[0]), [s1, n1], [s2, n2]])


def _build_module():
    nc = bacc.Bacc(None, target_bir_lowering=False, debug=False)

    x4_d = nc.dram_tensor("x4", [SPC, 4, 128, QS], _f16, kind="ExternalInput")
    wc_d = nc.dram_tensor("wc", [128, 4, 512], _f16, kind="ExternalInput")
    ws_d = nc.dram_tensor("ws", [128, 4, 512], _f16, kind="ExternalInput")
    wny_d = nc.dram_tensor("wny", [128, 4], _f16, kind="ExternalInput")
    tmpl_d = nc.dram_tensor("tmpl", [1, 128], _f32, kind="ExternalInput")
    ident_d = nc.dram_tensor("ident", [128, 128], _f16, kind="ExternalInput")
    o_d = nc.dram_tensor("o", [SPC, NBINS, T], _f16, kind="ExternalOutput")

    with tile.TileContext(nc) as tc:
        with ExitStack() as ctx:
            consts = ctx.enter_context(tc.tile_pool(name="consts", bufs=1))
            slabs = ctx.enter_context(tc.tile_pool(name="slabs", bufs=2))
            uvs = ctx.enter_context(tc.tile_pool(name="uvs", bufs=2))
            stats = ctx.enter_context(tc.tile_pool(name="stats", bufs=2))
            tmps = ctx.enter_context(tc.tile_pool(name="tmps", bufs=3))
            outs = ctx.enter_context(tc.tile_pool(name="outs", bufs=2))
            pmain = ctx.enter_context(tc.tile_pool(name="pmain", bufs=1, space="PSUM"))
            psmall = ctx.enter_context(tc.tile_pool(name="psmall", bufs=1, space="PSUM"))

            wc_sb = consts.tile([128, 4, 512], _f16)
            nc.sync.dma_start(out=wc_sb, in_=wc_d[:, :, :])
            ws_sb = consts.tile([128, 4, 512], _f16)
            nc.sync.dma_start(out=ws_sb, in_=ws_d[:, :, :])
            wny_sb = consts.tile([128, 4], _f16)
            nc.sync.dma_start(out=wny_sb, in_=wny_d[:, :])
            tmplP = consts.tile([1, 128], _f32)
            nc.sync.dma_start(out=tmplP, in_=tmpl_d[:, :])
            ident_sb = consts.tile([128, 128], _f16)
            nc.sync.dma_start(out=ident_sb, in_=ident_d[:, :])

            for s in range(SPC):
                xa = slabs.tile([128, QS], _f16, tag="xa", name=f"xa{s}")
                nc.sync.dma_start(out=xa, in_=x4_d[s, 0, :, :])
                xb = slabs.tile([128, QS], _f16, tag="xb", name=f"xb{s}")
                nc.sync.dma_start(out=xb, in_=x4_d[s, 1, :, :])
                xrevA = slabs.tile([128, QS], _f16, tag="xrA", name=f"xrA{s}")
                nc.sync.dma_start(out=xrevA, in_=x4_d[s, 2, :, :])
                xrevB = slabs.tile([128, QS], _f16, tag="xrB", name=f"xrB{s}")
                nc.sync.dma_start(out=xrevB, in_=x4_d[s, 3, :, :])

                # fold: u/v [128, 8v+4par+c], even par=0 from xa/xrevA,
                # odd par=1 from xb/xrevB (odd v=500 col is finite junk);
                # rev slabs are host-gathered in forward column order so all
                # strides are positive
                u = uvs.tile([128, 8 * NV], _f16, tag="u", name=f"u{s}")
                v = uvs.tile([128, 8 * NV], _f16, tag="v", name=f"v{s}")
                for par, fwd, rev, off_f in ((0, xa, xrevA, 0), (1, xb, xrevB, 2)):
                    in0 = _ap3(fwd, off_f, 5, NV, 1, 4)
                    in1 = _ap3(rev, 0, 5, NV, 1, 4)
                    uo = _ap3(u, 4 * par, 8, NV, 1, 4)
                    vo = _ap3(v, 4 * par, 8, NV, 1, 4)
                    nc.vector.tensor_add(out=uo, in0=in0, in1=in1)
                    nc.vector.tensor_sub(out=vo, in0=in0, in1=in1)

                # mean chain: sum(xa[:, 4:2504]) == sum(x) up to two edge
                # samples (~1e-5 relative on mu — far below tolerance);
                # bias = tmpl * S (tmpl carries /L and sqrt(lambd) scaling)
                sS = stats.tile([1, 1], _f32, tag="sS", name=f"sS{s}")
                nc.gpsimd.reduce_sum(out=sS, in_=xa[:, 4:2504],
                                     axis=mybir.AxisListType.XYZWC)
                # one shared PSUM bank per sample: pny cols 0:8, pT 128:256,
                # pbias col 256
                misc = psmall.tile([128, 512], _f32, tag="misc", bufs=2,
                                   name=f"misc{s}")
                nc.tensor.matmul(misc[:, 256:257], lhsT=tmplP[:, :],
                                 rhs=sS[:, :], start=True, stop=True)
                bias_sb = stats.tile([128, 1], _f32, tag="bias", name=f"bias{s}")
                nc.scalar.activation(out=bias_sb, in_=misc[:, 256:257],
                                     func=mybir.ActivationFunctionType.Copy)

                # Nyquist bin: data-stationary chains, frames on out partitions
                pny = misc[:, 0:8]
                nc.vector.memset(misc[:, 7:8], 0.0)  # fg7 pads partitions 106+
                ubase = u[:, 0:1]
                for fg in range(8):
                    nv = 64 if fg < 7 else 53
                    m = 2 * nv
                    for c in range(4):
                        lhsT = AP(ubase.tensor, ubase.offset + 8 * 64 * fg + c,
                                  [list(ubase.ap[0]), [8, nv], [4, 2]])
                        nc.tensor.matmul(pny[0:m, fg:fg + 1], lhsT=lhsT,
                                         rhs=wny_sb[:, c:c + 1],
                                         start=(c == 0), stop=(c == 3))
                syT = stats.tile([128, 8], _f16, tag="syT", name=f"syT{s}")
                nc.scalar.activation(out=syT, in_=pny,
                                     func=mybir.ActivationFunctionType.Square)
                pT = misc[0:8, 128:256]
                nc.tensor.matmul(pT, lhsT=syT[:, :], rhs=ident_sb[:, :],
                                 start=True, stop=True)
                nyrow = stats.tile([8, 128], _f16, tag="nyrow", name=f"ny{s}")
                nc.scalar.activation(out=nyrow, in_=pT,
                                     func=mybir.ActivationFunctionType.Ln, bias=1.0)
                nc.sync.dma_start(
                    out=o_d[s, 512:513, 0:896].rearrange("one (fg j) -> (one fg) j", fg=7),
                    in_=nyrow[0:7, :],
                )
                nc.sync.dma_start(out=o_d[s, 512:513, 896:1001], in_=nyrow[7:8, 0:105])

                osb = outs.tile([128, 4 * 1002], _f16, tag="osb", name=f"osb{s}")
                for kb in range(4):
                    pc = pmain.tile([128, 1024], _f32, tag="pc", bufs=2, name=f"pc{s}_{kb}")
                    ps_ = pmain.tile([128, 1024], _f32, tag="ps", bufs=1, name=f"ps{s}_{kb}")
                    for par in range(2):
                        for c in range(4):
                            rhs = _ap3(u, 4 * par + c, 8, NV, 1, 1).squeeze(2)
                            nc.tensor.matmul(
                                pc[:, 512 * par:512 * par + NV],
                                lhsT=wc_sb[:, c, 128 * kb:128 * kb + 128],
                                rhs=rhs, start=(c == 0), stop=(c == 3))
                    for par in range(2):
                        for c in range(4):
                            rhs = _ap3(v, 4 * par + c, 8, NV, 1, 1).squeeze(2)
                            nc.tensor.matmul(
                                ps_[:, 512 * par:512 * par + NV],
                                lhsT=ws_sb[:, c, 128 * kb:128 * kb + 128],
                                rhs=rhs, start=(c == 0), stop=(c == 3))
                    # strided PSUM views skip the bank-pad junk cols
                    pcv = _ap3(pc, 0, 512, 2, 1, NV)
                    psv = _ap3(ps_, 0, 512, 2, 1, NV)
                    t1 = tmps.tile([128, 2 * NV], _f16, tag="t1", name=f"t1{s}_{kb}")
                    t1v = _ap3(t1, 0, NV, 2, 1, NV)
                    nc.scalar.activation(
                        out=t1v, in_=pcv, func=mybir.ActivationFunctionType.Square,
                        bias=(bias_sb[:, 0:1] if kb == 0 else 0.0))
                    t2 = tmps.tile([128, 2 * NV], _f16, tag="t2", name=f"t2{s}_{kb}")
                    t2v = _ap3(t2, 0, NV, 2, 1, NV)
                    if kb in (0, 2):  # ACT square (DVE can't dual-read PSUM)
                        nc.scalar.activation(
                            out=t2v, in_=psv,
                            func=mybir.ActivationFunctionType.Square)
                    else:  # DVE: copy PSUM->fp16, then 2x-mode self-mult
                        cp = tmps.tile([128, 2 * NV], _f16, tag="cp",
                                       name=f"cp{s}_{kb}")
                        cpv = _ap3(cp, 0, NV, 2, 1, NV)
                        nc.vector.tensor_copy(out=cpv, in_=psv)
                        nc.vector.tensor_mul(out=t2[:, :], in0=cp[:, :],
                                             in1=cp[:, :])
                    pw = tmps.tile([128, 2 * NV], _f16, tag="pw", name=f"pw{s}_{kb}")
                    eng = nc.vector if kb < 2 else nc.gpsimd
                    eng.tensor_add(out=pw[:, :], in0=t1[:, :], in1=t2[:, :])
                    # ln1p -> t-interleaved f32 out block
                    obase = osb[:, 0:1]
                    oap = AP(obase.tensor, obase.offset + 1002 * kb,
                             [list(obase.ap[0]), [1, 2], [2, NV]])
                    pwv = _ap3(pw, 0, NV, 2, 1, NV)
                    nc.scalar.activation(out=oap, in_=pwv,
                                         func=mybir.ActivationFunctionType.Ln, bias=1.0)
                    # drain this kb's 128 bins while later kbs compute
                    nc.sync.dma_start(
                        out=o_d[s, 128 * kb:128 * kb + 128, :],
                        in_=osb[:, 1002 * kb:1002 * kb + T],
                    )

    nc.compile()
    return nc


def _host_prepare(x, lambd):
    """Build per-core slab inputs + folded DFT matrices (fp16)."""
    x = np.ascontiguousarray(x, dtype=np.float32)
    lam = float(np.asarray(lambd, dtype=np.float32))
    sq = np.sqrt(abs(lam)) if lam != 0 else 1.0

    xp = np.concatenate(
        [x[:, PAD:0:-1], x, x[:, L - 2: L - 2 - PAD: -1]], axis=1
    )  # [B, LP]
    nq = 128 * QS  # 320768 <= LP

    def slab(src, off):
        return np.ascontiguousarray(
            src[:, off:off + nq].reshape(B, QS, 128).transpose(0, 2, 1)
        ).astype(np.float16)

    xa = slab(xp, 1)   # xp[128q+p+1]
    xb = slab(xp, 65)  # xp[128q+p+65]

    # reversed-operand slabs, gathered in forward column order:
    # xrev*[p, 5v+c] = xp[base + 640v - 128c - p]
    vv = np.arange(NV)
    cc = np.arange(4)
    pp = np.arange(128)
    idx = (640 * vv[:, None, None] - 128 * cc[None, :, None]
           - pp[None, None, :])  # [NV, 4, 128]
    cols = (5 * vv[:, None] + cc[None, :]).ravel()  # 2004 used columns

    def revslab(base):
        iz = np.clip(base + idx, 0, LP - 1)
        vals = xp[:, iz]  # [B, NV, 4, 128]
        out = np.zeros((B, 128, QS), dtype=np.float16)
        out[:, :, cols] = vals.transpose(0, 3, 1, 2).reshape(B, 128, 4 * NV)
        return out

    xrevA = revslab(1023)  # xp[640v + 1023 - 128c - p]
    xrevB = revslab(1343)  # xp[640v + 1343 - 128c - p]
    x4 = np.ascontiguousarray(np.stack([xa, xb, xrevA, xrevB], axis=1))

    n = np.arange(1, 513, dtype=np.float64)  # contraction slots 1..512
    win = 0.5 * (1.0 - np.cos(2.0 * np.pi * n / N_FFT))
    k = np.arange(512, dtype=np.float64)
    ang = 2.0 * np.pi * np.outer(n, k) / N_FFT
    wc64 = sq * win[:, None] * np.cos(ang)
    ws64 = sq * win[:, None] * np.sin(ang)
    wc64[511, :] *= 0.5  # u[512] = 2*y[512]
    ws64[511, :] = 0.0
    wny64 = sq * win * np.cos(np.pi * n)
    wny64[511] = 0.5 * sq

    def to_pck(w):  # [512, nk] -> [128, 4, nk], slot n=128c+p+1
        return np.ascontiguousarray(
            w.reshape(4, 128, -1).transpose(1, 0, 2)).astype(np.float16)

    wc = to_pck(wc64)
    ws = to_pck(ws64)
    wny = np.ascontiguousarray(
        wny64.reshape(4, 128).transpose(1, 0)).astype(np.float16)
    tmpl = np.zeros((1, 128), dtype=np.float32)
    tmpl[0, 0] = -512.0 * sq / L
    tmpl[0, 1] = 256.0 * sq / L
    ident = np.eye(128, dtype=np.float16)
    return x4, wc, ws, wny, tmpl, ident


def _in_maps(x4, wc, ws, wny, tmpl, ident):
    maps = []
    for c in range(NCORES):
        sl = slice(c * SPC, (c + 1) * SPC)
        maps.append({
            "x4": np.ascontiguousarray(x4[sl]),
            "wc": wc, "ws": ws, "wny": wny, "tmpl": tmpl, "ident": ident,
        })
    return maps


def kernel(x, lambd):
    from concourse.bass_utils import run_bass_kernel_spmd

    prep = _host_prepare(x, lambd)
    nc = _build_module()
    res = run_bass_kernel_spmd(nc, _in_maps(*prep), core_ids=list(range(NCORES)))
    out = np.concatenate([res.results[c]["o"] for c in range(NCORES)], axis=0)
    return out.astype(np.float32)


if __name__ == "__main__":
    rng = np.random.default_rng(0)
    x = rng.standard_normal((B, L), dtype=np.float32)
    out = kernel(x, np.float32(5.0))
    print(out.shape, out.dtype, out[0, :3, :3])



# revision 24
# speedup vs baseline: 1.2336x; 1.1954x over previous
"""Power-STFT kernel for Trainium2 (8 NeuronCores, data-parallel over batch).

Computes, for x [32, 320000] and scalar lambd:
    x <- x - mean(x, axis=1)
    power-STFT (n_fft=1024, hop=320, periodic Hann, center reflect pad)
    out = log1p(lambd * power)   -> [32, 513, 1001] fp32

Strategy per core (4 batch samples per core), v2 — folded DFT in fp16:
  - Window/trig symmetry: win(1024-n) = win(n), cos sym / sin antisym about
    n=512, so the windowed DFT reduces to a length-512 contraction over
      u_t[n] = y_t[n] + y_t[1024-n],  v_t[n] = y_t[n] - y_t[1024-n]
    (n = 1..512; u[512] = 2*y[512] absorbed with half weight; n=0 has
    win(0) = 0). This HALVES the tensor-engine work vs the direct 8-chunk
    form: 4 contraction chunks of 128 per trig.
  - All matmul inputs are fp16 (full PE rate, 1 row/cycle). Host ships four
    slab views of the reflect-padded signal so every fold operand is
    partition-aligned: xa/xb (forward, offsets +1/+65 so contraction slot
    (c,p) = sample 128c+p+1) and xrevA/xrevB (reversed: xp[321023-i] and
    xp[320959-i]) for even/odd frames. u/v are built by 4 DVE adds/subs per
    sample (fp16 2x mode) into a [128, v*8+par*4+c] layout whose matmul rhs
    slices are [128, stride-8 x 501].
  - Nyquist bin (512) rides data-stationary matmuls: lhsT = u frames chunk
    (M = 128 frames), rhs = folded (-1)^n window column (F=1) accumulated
    over 4 chunks into a [128 frames, 8 fgroup] PSUM tile; squared, PE-
    transposed via identity matmul, Ln'd, and DMA'd as output row 512.
  - Mean removal: periodic-Hann DFT of a constant is exactly [512, -256]
    at bins 0/1 (real), zero elsewhere. So demeaning == biasing bins 0/1
    of the cos part: bias vec = mu * [-512, 256, 0...] * sqrt(lambd),
    applied as the per-partition bias of the kb=0 cos ACT Square. mu comes
    from a GPSIMD column-reduce of xa + a 3-matmul partition-sum with
    edge fixups.
  - Epilogue per (kb): cos^2 on ACT (Square, PSUM strided view skipping
    bank-pad junk), sin^2 on DVE (scalar_tensor_tensor (ps*1)*ps), power
    add split DVE/GPSIMD, log1p via ACT Ln(power + 1) writing the t-
    interleaved fp32 out tile; one 2MB DMA per sample for bins 0..511.
  - sqrt(lambd) folded into the DFT matrices so power is pre-scaled.
"""

import sys

sys.path.insert(0, "/opt/trn_rl_repo")

import numpy as np

import concourse.bacc as bacc
import concourse.bass as bass
import concourse.mybir as mybir
import concourse.tile as tile
from concourse.ap import AP
from contextlib import ExitStack

N_FFT = 1024
HOP = 320
L = 320000
PAD = N_FFT // 2  # 512
LP = L + 2 * PAD  # 321024
B = 32
NCORES = 8
SPC = B // NCORES  # 4 samples per core
T = 1 + L // HOP  # 1001 frames
NV = 501  # even-frame count; odd frames use 500 + 1 junk col
QS = 2506  # slab columns
NBINS = 513

_f32 = mybir.dt.float32
_f16 = mybir.dt.float16


def _ap3(t, col_off, s1, n1, s2, n2):
    """[128, n1, n2] AP on tile t with free strides (s1, s2) from col_off."""
    base = t[:, 0:1]
    return AP(base.tensor, base.offset + col_off,
              [list(base.ap[0]), [s1, n1], [s2, n2]])


def _ap4(t, col_off, s1, n1, s2, n2, s3, n3):
    """[128, n1, n2, n3] AP on tile t with free strides (s1, s2, s3)."""
    base = t[:, 0:1]
    return AP(base.tensor, base.offset + col_off,
              [list(base.ap[0]), [s1, n1], [s2, n2], [s3, n3]])


def _build_module():
    nc = bacc.Bacc(None, target_bir_lowering=False, debug=False)

    x4_d = nc.dram_tensor("x4", [SPC, 4, 128, QS], _f16, kind="ExternalInput")
    wc_d = nc.dram_tensor("wc", [128, 4, 512], _f16, kind="ExternalInput")
    ws_d = nc.dram_tensor("ws", [128, 4, 512], _f16, kind="ExternalInput")
    wny_d = nc.dram_tensor("wny", [128, 4], _f16, kind="ExternalInput")
    tmpl_d = nc.dram_tensor("tmpl", [1, 128], _f32, kind="ExternalInput")
    o_d = nc.dram_tensor("o", [SPC, NBINS, T], _f16, kind="ExternalOutput")

    with tile.TileContext(nc) as tc:
        with ExitStack() as ctx:
            consts = ctx.enter_context(tc.tile_pool(name="consts", bufs=1))
            slabs = ctx.enter_context(tc.tile_pool(name="slabs", bufs=2))
            uvs = ctx.enter_context(tc.tile_pool(name="uvs", bufs=2))
            stats = ctx.enter_context(tc.tile_pool(name="stats", bufs=2))
            tmps = ctx.enter_context(tc.tile_pool(name="tmps", bufs=3))
            outs = ctx.enter_context(tc.tile_pool(name="outs", bufs=2))
            pmain = ctx.enter_context(tc.tile_pool(name="pmain", bufs=1, space="PSUM"))

            # v-range halves: (v0, nvh) — h0 sized so first-chunk DMAs
            # (cols < CSPLIT) cover all its fold reads
            HALVES = ((0, 251), (251, 250))
            CSPLIT = 1280

            slab_tiles = []
            for s in range(SPC):
                tiles = []
                for i, tg in enumerate(("xa", "xb", "xrA", "xrB")):
                    t = slabs.tile([128, QS], _f16, tag=tg, name=f"{tg}{s}")
                    tiles.append(t)
                slab_tiles.append(tiles)
                # first sample's first-half chunks lead everything
                if s == 0:
                    for i, t in enumerate(tiles):
                        nc.sync.dma_start(out=t[:, 0:CSPLIT],
                                          in_=x4_d[s, i, :, 0:CSPLIT])

            wc_sb = consts.tile([128, 4, 512], _f16)
            nc.sync.dma_start(out=wc_sb, in_=wc_d[:, :, :])
            ws_sb = consts.tile([128, 4, 512], _f16)
            nc.sync.dma_start(out=ws_sb, in_=ws_d[:, :, :])
            wny_sb = consts.tile([128, 4], _f16)
            nc.sync.dma_start(out=wny_sb, in_=wny_d[:, :])
            tmplP = consts.tile([1, 128], _f32)
            nc.sync.dma_start(out=tmplP, in_=tmpl_d[:, :])

            for s in range(SPC):
                tiles = slab_tiles[s]
                if s == 0:
                    for i, t in enumerate(tiles):
                        nc.sync.dma_start(out=t[:, CSPLIT:QS],
                                          in_=x4_d[s, i, :, CSPLIT:QS])
                else:
                    for i, t in enumerate(tiles):
                        nc.sync.dma_start(out=t[:, 0:CSPLIT],
                                          in_=x4_d[s, i, :, 0:CSPLIT])
                        nc.sync.dma_start(out=t[:, CSPLIT:QS],
                                          in_=x4_d[s, i, :, CSPLIT:QS])
                xa, xb, xrevA, xrevB = tiles

                # fold: u/v [128, 8v+4par+c], even par=0 from xa/xrevA,
                # odd par=1 from xb/xrevB (odd v=500 col is finite junk);
                # rev slabs are host-gathered in forward column order so all
                # strides are positive
                u = uvs.tile([128, 8 * NV], _f16, tag="u", name=f"u{s}")
                v = uvs.tile([128, 8 * NV], _f16, tag="v", name=f"v{s}")
                for v0, nvh in HALVES:
                    for par, fwd, rev, off_f in ((0, xa, xrevA, 0),
                                                 (1, xb, xrevB, 2)):
                        in0 = _ap3(fwd, off_f + 5 * v0, 5, nvh, 1, 4)
                        in1 = _ap3(rev, 5 * v0, 5, nvh, 1, 4)
                        uo = _ap3(u, 4 * par + 8 * v0, 8, nvh, 1, 4)
                        vo = _ap3(v, 4 * par + 8 * v0, 8, nvh, 1, 4)
                        nc.vector.tensor_add(out=uo, in0=in0, in1=in1)
                        nc.vector.tensor_sub(out=vo, in0=in0, in1=in1)

                # mean chain: sum(xa[:, 4:2504]) == sum(x) up to two edge
                # samples (~1e-5 relative on mu — far below tolerance);
                # bias[k] = sum_p sum_j tmpl[p,k]*sS2[p,j] = tmplvals[k] * S
                # (tmpl carries /L and sqrt(lambd) scaling)
                sS2 = stats.tile([1, 2], _f32, tag="sS", name=f"sS{s}")
                nc.gpsimd.reduce_sum(out=sS2[:, 0:1], in_=xa[:, 4:CSPLIT],
                                     axis=mybir.AxisListType.XYZWC)
                nc.gpsimd.reduce_sum(out=sS2[:, 1:2], in_=xa[:, CSPLIT:2504],
                                     axis=mybir.AxisListType.XYZWC)
                # misc shares the pc buffer rotation: pny cols 0:8, pbias 256
                misc = pmain.tile([128, 1024], _f32, tag="pc", bufs=2,
                                  name=f"misc{s}")
                nc.tensor.matmul(misc[:, 256:257], lhsT=tmplP[:, :],
                                 rhs=sS2[:, 0:1], start=True, stop=False)
                nc.tensor.matmul(misc[:, 256:257], lhsT=tmplP[:, :],
                                 rhs=sS2[:, 1:2], start=False, stop=True)
                bias_sb = stats.tile([128, 1], _f32, tag="bias", name=f"bias{s}")
                nc.vector.tensor_copy(out=bias_sb, in_=misc[:, 256:257])

                # Nyquist bin: data-stationary chains, frames on out partitions
                pny = misc[:, 0:8]
                nc.vector.memset(misc[:, 7:8], 0.0)  # fg7 pads partitions 106+
                ubase = u[:, 0:1]
                for fg in range(8):
                    nv = 64 if fg < 7 else 53
                    m = 2 * nv
                    for c in range(4):
                        lhsT = AP(ubase.tensor, ubase.offset + 8 * 64 * fg + c,
                                  [list(ubase.ap[0]), [8, nv], [4, 2]])
                        nc.tensor.matmul(pny[0:m, fg:fg + 1], lhsT=lhsT,
                                         rhs=wny_sb[:, c:c + 1],
                                         start=(c == 0), stop=(c == 3))
                syT = stats.tile([128, 8], _f16, tag="syT", name=f"syT{s}")
                nc.scalar.activation(out=syT, in_=pny,
                                     func=mybir.ActivationFunctionType.Square)
                nyrow = stats.tile([128, 8], _f16, tag="nyrow", name=f"ny{s}")
                nc.scalar.activation(out=nyrow, in_=syT,
                                     func=mybir.ActivationFunctionType.Ln, bias=1.0)
                # frame t = 128*fg + p: transposing gather DMAs
                nc.sync.dma_start(
                    out=o_d[s, 512:513, 0:896].rearrange("one (fg p) -> p (one fg)", fg=7),
                    in_=nyrow[:, 0:7],
                )
                nc.sync.dma_start(
                    out=o_d[s, 512:513, 896:1001].rearrange("one p -> p one"),
                    in_=nyrow[0:105, 7:8])

                # pc/ps col layout: 512*h + 256*par + v' (one accumulation
                # group per (kb, h) = per bank, 2D (par, v') out AP);
                # t1/t2/cp/pw are quadrant-major packed: 502*h + 251*par + v'
                osb = outs.tile([128, 4 * 1002 + 2], _f16, tag="osb", name=f"osb{s}")
                for kb in range(4):
                    pc = pmain.tile([128, 1024], _f32, tag="pc", bufs=2, name=f"pc{s}_{kb}")
                    ps_ = pmain.tile([128, 1024], _f32, tag="ps", bufs=2, name=f"ps{s}_{kb}")
                    for h, (v0, nvh) in enumerate(HALVES):
                        for c in range(4):
                            rhs = _ap3(u, 8 * v0 + c, 4, 2, 8, nvh)
                            nc.tensor.matmul(
                                _ap3(pc, 512 * h, 256, 2, 1, nvh),
                                lhsT=wc_sb[:, c, 128 * kb:128 * kb + 128],
                                rhs=rhs, start=(c == 0), stop=(c == 3))
                        for c in range(4):
                            rhs = _ap3(v, 8 * v0 + c, 4, 2, 8, nvh)
                            nc.tensor.matmul(
                                _ap3(ps_, 512 * h, 256, 2, 1, nvh),
                                lhsT=ws_sb[:, c, 128 * kb:128 * kb + 128],
                                rhs=rhs, start=(c == 0), stop=(c == 3))
                    # strided PSUM views skip the bank-pad junk cols
                    pcv = _ap4(pc, 0, 512, 2, 256, 2, 1, 251)
                    psv = _ap4(ps_, 0, 512, 2, 256, 2, 1, 251)
                    t1 = tmps.tile([128, 1004], _f16, tag="t1", name=f"t1{s}_{kb}")
                    t1v = _ap4(t1, 0, 502, 2, 251, 2, 1, 251)
                    nc.scalar.activation(
                        out=t1v, in_=pcv, func=mybir.ActivationFunctionType.Square,
                        bias=(bias_sb[:, 0:1] if kb == 0 else 0.0))
                    t2 = tmps.tile([128, 1004], _f16, tag="t2", name=f"t2{s}_{kb}")
                    t2v = _ap4(t2, 0, 502, 2, 251, 2, 1, 251)
                    if kb in (0, 2):  # ACT square (DVE can't dual-read PSUM)
                        nc.scalar.activation(
                            out=t2v, in_=psv,
                            func=mybir.ActivationFunctionType.Square)
                    else:  # DVE: copy PSUM->fp16, then 2x-mode self-mult
                        cp = tmps.tile([128, 1004], _f16, tag="cp",
                                       name=f"cp{s}_{kb}")
                        cpv = _ap4(cp, 0, 502, 2, 251, 2, 1, 251)
                        nc.vector.tensor_copy(out=cpv, in_=psv)
                        nc.vector.tensor_mul(out=t2[:, :], in0=cp[:, :],
                                             in1=cp[:, :])
                    pw = tmps.tile([128, 1004], _f16, tag="pw", name=f"pw{s}_{kb}")
                    nc.vector.tensor_add(out=pw[:, :], in0=t1[:, :], in1=t2[:, :])
                    # ln1p -> t-interleaved f16 out block (t = 502h + 2v' + par)
                    obase = osb[:, 0:1]
                    oap = AP(obase.tensor, obase.offset + 1002 * kb,
                             [list(obase.ap[0]), [502, 2], [1, 2], [2, 251]])
                    pwv = _ap4(pw, 0, 502, 2, 251, 2, 1, 251)
                    nc.scalar.activation(out=oap, in_=pwv,
                                         func=mybir.ActivationFunctionType.Ln, bias=1.0)
                    # drain this kb's 128 bins while later kbs compute
                    nc.sync.dma_start(
                        out=o_d[s, 128 * kb:128 * kb + 128, :],
                        in_=osb[:, 1002 * kb:1002 * kb + T],
                    )

    nc.compile()
    return nc


def _host_prepare(x, lambd):
    """Build per-core slab inputs + folded DFT matrices (fp16)."""
    x = np.ascontiguousarray(x, dtype=np.float32)
    lam = float(np.asarray(lambd, dtype=np.float32))
    sq = np.sqrt(abs(lam)) if lam != 0 else 1.0

    xp = np.concatenate(
        [x[:, PAD:0:-1], x, x[:, L - 2: L - 2 - PAD: -1]], axis=1
    )  # [B, LP]
    nq = 128 * QS  # 320768 <= LP

    def slab(src, off):
        return np.ascontiguousarray(
            src[:, off:off + nq].reshape(B, QS, 128).transpose(0, 2, 1)
        ).astype(np.float16)

    xa = slab(xp, 1)   # xp[128q+p+1]
    xb = slab(xp, 65)  # xp[128q+p+65]

    # reversed-operand slabs, gathered in forward column order:
    # xrev*[p, 5v+c] = xp[base + 640v - 128c - p]
    vv = np.arange(NV)
    cc = np.arange(4)
    pp = np.arange(128)
    idx = (640 * vv[:, None, None] - 128 * cc[None, :, None]
           - pp[None, None, :])  # [NV, 4, 128]
    cols = (5 * vv[:, None] + cc[None, :]).ravel()  # 2004 used columns

    def revslab(base):
        iz = np.clip(base + idx, 0, LP - 1)
        vals = xp[:, iz]  # [B, NV, 4, 128]
        out = np.zeros((B, 128, QS), dtype=np.float16)
        out[:, :, cols] = vals.transpose(0, 3, 1, 2).reshape(B, 128, 4 * NV)
        return out

    xrevA = revslab(1023)  # xp[640v + 1023 - 128c - p]
    xrevB = revslab(1343)  # xp[640v + 1343 - 128c - p]
    x4 = np.ascontiguousarray(np.stack([xa, xb, xrevA, xrevB], axis=1))

    n = np.arange(1, 513, dtype=np.float64)  # contraction slots 1..512
    win = 0.5 * (1.0 - np.cos(2.0 * np.pi * n / N_FFT))
    k = np.arange(512, dtype=np.float64)
    ang = 2.0 * np.pi * np.outer(n, k) / N_FFT
    wc64 = sq * win[:, None] * np.cos(ang)
    ws64 = sq * win[:, None] * np.sin(ang)
    wc64[511, :] *= 0.5  # u[512] = 2*y[512]
    ws64[511, :] = 0.0
    wny64 = sq * win * np.cos(np.pi * n)
    wny64[511] = 0.5 * sq

    def to_pck(w):  # [512, nk] -> [128, 4, nk], slot n=128c+p+1
        return np.ascontiguousarray(
            w.reshape(4, 128, -1).transpose(1, 0, 2)).astype(np.float16)

    wc = to_pck(wc64)
    ws = to_pck(ws64)
    wny = np.ascontiguousarray(
        wny64.reshape(4, 128).transpose(1, 0)).astype(np.float16)
    tmpl = np.zeros((1, 128), dtype=np.float32)
    tmpl[0, 0] = -512.0 * sq / L
    tmpl[0, 1] = 256.0 * sq / L
    return x4, wc, ws, wny, tmpl


def _in_maps(x4, wc, ws, wny, tmpl):
    maps = []
    for c in range(NCORES):
        sl = slice(c * SPC, (c + 1) * SPC)
        maps.append({
            "x4": np.ascontiguousarray(x4[sl]),
            "wc": wc, "ws": ws, "wny": wny, "tmpl": tmpl,
        })
    return maps


def kernel(x, lambd):
    from concourse.bass_utils import run_bass_kernel_spmd

    prep = _host_prepare(x, lambd)
    nc = _build_module()
    res = run_bass_kernel_spmd(nc, _in_maps(*prep), core_ids=list(range(NCORES)))
    out = np.concatenate([res.results[c]["o"] for c in range(NCORES)], axis=0)
    return out.astype(np.float32)


if __name__ == "__main__":
    rng = np.random.default_rng(0)
    x = rng.standard_normal((B, L), dtype=np.float32)
    out = kernel(x, np.float32(5.0))
    print(out.shape, out.dtype, out[0, :3, :3])



# revision 45
# speedup vs baseline: 1.2787x; 1.0366x over previous
"""Power-STFT kernel for Trainium2 (8 NeuronCores, data-parallel over batch).

Computes, for x [32, 320000] and scalar lambd:
    x <- x - mean(x, axis=1)
    power-STFT (n_fft=1024, hop=320, periodic Hann, center reflect pad)
    out = log1p(lambd * power)   -> [32, 513, 1001] fp32

Strategy per core (4 batch samples per core), v2 — folded DFT in fp16:
  - Window/trig symmetry: win(1024-n) = win(n), cos sym / sin antisym about
    n=512, so the windowed DFT reduces to a length-512 contraction over
      u_t[n] = y_t[n] + y_t[1024-n],  v_t[n] = y_t[n] - y_t[1024-n]
    (n = 1..512; u[512] = 2*y[512] absorbed with half weight; n=0 has
    win(0) = 0). This HALVES the tensor-engine work vs the direct 8-chunk
    form: 4 contraction chunks of 128 per trig.
  - All matmul inputs are fp16 (full PE rate, 1 row/cycle). Host ships four
    slab views of the reflect-padded signal so every fold operand is
    partition-aligned: xa/xb (forward, offsets +1/+65 so contraction slot
    (c,p) = sample 128c+p+1) and xrevA/xrevB (reversed: xp[321023-i] and
    xp[320959-i]) for even/odd frames. u/v are built by 4 DVE adds/subs per
    sample (fp16 2x mode) into a [128, v*8+par*4+c] layout whose matmul rhs
    slices are [128, stride-8 x 501].
  - Nyquist bin (512) rides data-stationary matmuls: lhsT = u frames chunk
    (M = 128 frames), rhs = folded (-1)^n window column (F=1) accumulated
    over 4 chunks into a [128 frames, 8 fgroup] PSUM tile; squared, PE-
    transposed via identity matmul, Ln'd, and DMA'd as output row 512.
  - Mean removal: periodic-Hann DFT of a constant is exactly [512, -256]
    at bins 0/1 (real), zero elsewhere. So demeaning == biasing bins 0/1
    of the cos part: bias vec = mu * [-512, 256, 0...] * sqrt(lambd),
    applied as the per-partition bias of the kb=0 cos ACT Square. mu comes
    from a GPSIMD column-reduce of xa + a 3-matmul partition-sum with
    edge fixups.
  - Epilogue per (kb): cos^2 on ACT (Square, PSUM strided view skipping
    bank-pad junk), sin^2 on DVE (scalar_tensor_tensor (ps*1)*ps), power
    add split DVE/GPSIMD, log1p via ACT Ln(power + 1) writing the t-
    interleaved fp32 out tile; one 2MB DMA per sample for bins 0..511.
  - sqrt(lambd) folded into the DFT matrices so power is pre-scaled.
"""

import sys

sys.path.insert(0, "/opt/trn_rl_repo")

import numpy as np

import concourse.bacc as bacc
import concourse.bass as bass
import concourse.mybir as mybir
import concourse.tile as tile
from concourse.ap import AP
from contextlib import ExitStack

N_FFT = 1024
HOP = 320
L = 320000
PAD = N_FFT // 2  # 512
LP = L + 2 * PAD  # 321024
B = 32
NCORES = 8
SPC = B // NCORES  # 4 samples per core
T = 1 + L // HOP  # 1001 frames
NV = 501  # even-frame count; odd frames use 500 + 1 junk col
QS = 2506  # slab columns
NBINS = 513

_f32 = mybir.dt.float32
_f16 = mybir.dt.float16


def _ap3(t, col_off, s1, n1, s2, n2):
    """[128, n1, n2] AP on tile t with free strides (s1, s2) from col_off."""
    base = t[:, 0:1]
    return AP(base.tensor, base.offset + col_off,
              [list(base.ap[0]), [s1, n1], [s2, n2]])


def _ap4(t, col_off, s1, n1, s2, n2, s3, n3):
    """[128, n1, n2, n3] AP on tile t with free strides (s1, s2, s3)."""
    base = t[:, 0:1]
    return AP(base.tensor, base.offset + col_off,
              [list(base.ap[0]), [s1, n1], [s2, n2], [s3, n3]])


def _build_module():
    nc = bacc.Bacc(None, target_bir_lowering=False, debug=False)

    x4_d = nc.dram_tensor("x4", [SPC, 4, 128, QS], _f16, kind="ExternalInput")
    wc_d = nc.dram_tensor("wc", [4, 128, 4, 128], _f16, kind="ExternalInput")
    ws_d = nc.dram_tensor("ws", [4, 128, 4, 128], _f16, kind="ExternalInput")
    wny_d = nc.dram_tensor("wny", [128, 4], _f16, kind="ExternalInput")
    tmpl_d = nc.dram_tensor("tmpl", [128, 1], _f32, kind="ExternalInput")
    o_d = nc.dram_tensor("o", [SPC, NBINS, T], _f16, kind="ExternalOutput")

    with tile.TileContext(nc) as tc:
        with ExitStack() as ctx:
            consts = ctx.enter_context(tc.tile_pool(name="consts", bufs=1))
            slabs = ctx.enter_context(tc.tile_pool(name="slabs", bufs=2))
            uvs = ctx.enter_context(tc.tile_pool(name="uvs", bufs=3))
            stats = ctx.enter_context(tc.tile_pool(name="stats", bufs=2))
            tmps = ctx.enter_context(tc.tile_pool(name="tmps", bufs=3))
            outs = ctx.enter_context(tc.tile_pool(name="outs", bufs=2))
            pmain = ctx.enter_context(tc.tile_pool(name="pmain", bufs=1, space="PSUM"))

            # v-range halves: (v0, nvh) — h0 sized so first-chunk DMAs
            # (cols < CSPLIT) cover all its fold reads
            HALVES = ((0, 251), (251, 250))
            CSPLIT = 1280

            slab_tiles = []
            for s in range(SPC):
                tiles = []
                for i, tg in enumerate(("xa", "xb", "xrA", "xrB")):
                    t = slabs.tile([128, QS], _f16, tag=tg, name=f"{tg}{s}")
                    tiles.append(t)
                slab_tiles.append(tiles)
                # first sample's first-half chunks lead everything
                if s == 0:
                    for i, t in enumerate(tiles):
                        nc.sync.dma_start(out=t[:, 0:CSPLIT],
                                          in_=x4_d[s, i, :, 0:CSPLIT])

            # weights kb-major so kb0's slices land right after the lead
            # chunks; s0's tail chunks interleave between kb0 and kb1-3
            wc_sb = consts.tile([128, 4, 4, 128], _f16)  # [p, kb, c, k]
            ws_sb = consts.tile([128, 4, 4, 128], _f16)
            nc.sync.dma_start(out=wc_sb[:, 0], in_=wc_d[0])
            nc.sync.dma_start(out=ws_sb[:, 0], in_=ws_d[0])
            for i, t in enumerate(slab_tiles[0]):
                nc.sync.dma_start(out=t[:, CSPLIT:QS],
                                  in_=x4_d[0, i, :, CSPLIT:QS])
            for kb in range(1, 4):
                nc.sync.dma_start(out=wc_sb[:, kb], in_=wc_d[kb])
                nc.sync.dma_start(out=ws_sb[:, kb], in_=ws_d[kb])
            wny_sb = consts.tile([128, 4], _f16)
            nc.sync.dma_start(out=wny_sb, in_=wny_d[:, :])
            tmplP = consts.tile([128, 1], _f32)
            nc.sync.dma_start(out=tmplP, in_=tmpl_d[:, :])

            for s in range(SPC):
                tiles = slab_tiles[s]
                if s != 0:
                    for i, t in enumerate(tiles):
                        nc.sync.dma_start(out=t[:, 0:CSPLIT],
                                          in_=x4_d[s, i, :, 0:CSPLIT])
                        nc.sync.dma_start(out=t[:, CSPLIT:QS],
                                          in_=x4_d[s, i, :, CSPLIT:QS])
                xa, xb, xrevA, xrevB = tiles

                # fold: u/v [128, 8v+4par+c], even par=0 from xa/xrevA,
                # odd par=1 from xb/xrevB (odd v=500 col is finite junk);
                # rev slabs are host-gathered in forward column order so all
                # strides are positive
                u = uvs.tile([128, 8 * NV], _f16, tag="u", name=f"u{s}")
                v = uvs.tile([128, 8 * NV], _f16, tag="v", name=f"v{s}")
                for v0, nvh in HALVES:
                    for par, fwd, rev, off_f in ((0, xa, xrevA, 0),
                                                 (1, xb, xrevB, 2)):
                        in0 = _ap3(fwd, off_f + 5 * v0, 5, nvh, 1, 4)
                        in1 = _ap3(rev, 5 * v0, 5, nvh, 1, 4)
                        uo = _ap3(u, 4 * par + 8 * v0, 8, nvh, 1, 4)
                        vo = _ap3(v, 4 * par + 8 * v0, 8, nvh, 1, 4)
                        nc.vector.tensor_add(out=uo, in0=in0, in1=in1)
                        nc.vector.tensor_sub(out=vo, in0=in0, in1=in1)

                # mean chain: sum(xa[:, 4:2504]) == sum(x) up to two edge
                # samples (~1e-5 relative on mu — far below tolerance);
                # S = sS2[0]+sS2[1] broadcast to all partitions by a
                # stride-0 DMA, then bias = tmplcol * S on DVE (tmpl
                # carries /L and sqrt(lambd) scaling; rows 0/1 nonzero)
                sS2 = stats.tile([1, 2], _f32, tag="sS", name=f"sS{s}")
                nc.gpsimd.reduce_sum(out=sS2[:, 0:1], in_=xa[:, 4:CSPLIT],
                                     axis=mybir.AxisListType.XYZWC)
                nc.gpsimd.reduce_sum(out=sS2[:, 1:2], in_=xa[:, CSPLIT:2504],
                                     axis=mybir.AxisListType.XYZWC)
                Sb = stats.tile([1, 1], _f32, tag="Sb", name=f"Sb{s}")
                nc.vector.tensor_add(out=Sb, in0=sS2[:, 0:1], in1=sS2[:, 1:2])
                Sbrd = stats.tile([128, 1], _f32, tag="Sbrd", name=f"Sbrd{s}")
                nc.gpsimd.partition_broadcast(Sbrd[:, :], Sb[:, :])
                bias_sb = stats.tile([128, 1], _f32, tag="bias", name=f"bias{s}")
                nc.vector.tensor_mul(out=bias_sb, in0=tmplP[:, :], in1=Sbrd)

                def emit_ny(s=s, u=u):
                    # Nyquist bin: data-stationary chains, frames on out
                    # partitions; pny rides the ps rotation
                    misc = pmain.tile([128, 1024], _f32, tag="ps", bufs=2,
                                      name=f"misc{s}")
                    pny = misc[:, 0:8]
                    nc.vector.memset(misc[:, 7:8], 0.0)  # fg7 pad rows
                    ubase = u[:, 0:1]
                    for fg in range(8):
                        nv = 64 if fg < 7 else 53
                        m = 2 * nv
                        for c in range(4):
                            lhsT = AP(ubase.tensor,
                                      ubase.offset + 8 * 64 * fg + c,
                                      [list(ubase.ap[0]), [8, nv], [4, 2]])
                            nc.tensor.matmul(pny[0:m, fg:fg + 1], lhsT=lhsT,
                                             rhs=wny_sb[:, c:c + 1],
                                             start=(c == 0), stop=(c == 3))
                    syT = stats.tile([128, 8], _f16, tag="syT", name=f"syT{s}")
                    nc.scalar.activation(
                        out=syT, in_=pny,
                        func=mybir.ActivationFunctionType.Square)
                    nyrow = stats.tile([128, 8], _f16, tag="nyrow",
                                       name=f"ny{s}")
                    nc.scalar.activation(
                        out=nyrow, in_=syT,
                        func=mybir.ActivationFunctionType.Ln, bias=1.0)
                    # frame t = 128*fg + p: transposing gather DMAs
                    nc.sync.dma_start(
                        out=o_d[s, 512:513, 0:896].rearrange(
                            "one (fg p) -> p (one fg)", fg=7),
                        in_=nyrow[:, 0:7],
                    )
                    nc.sync.dma_start(
                        out=o_d[s, 512:513, 896:1001].rearrange(
                            "one p -> p one"),
                        in_=nyrow[0:105, 7:8])

                # pc/ps col layout: 512*h + 256*par + v' (one accumulation
                # group per (kb, h) = per bank, 2D (par, v') out AP);
                # t1/t2/cp/pw are quadrant-major packed: 502*h + 251*par + v'
                osb = outs.tile([128, 4 * 1002 + 2], _f16, tag="osb", name=f"osb{s}")
                if s == SPC - 1:
                    emit_ny()  # off the kernel tail: before kb0 (u is ready)
                for kb in range(4):
                    pc = pmain.tile([128, 1024], _f32, tag="pc", bufs=2, name=f"pc{s}_{kb}")
                    ps_ = pmain.tile([128, 1024], _f32, tag="ps", bufs=2, name=f"ps{s}_{kb}")
                    tail_kb = (s == SPC - 1 and kb == 3)
                    sin_dve = kb in (1, 3) or (s == SPC - 1 and kb == 2)
                    for h, (v0, nvh) in enumerate(HALVES):
                        # DVE-sin kbs: sin first so the DVE copy path starts
                        # while the cos matmuls still stream
                        trigs = (((ps_, ws_sb, v), (pc, wc_sb, u)) if sin_dve
                                 else ((pc, wc_sb, u), (ps_, ws_sb, v)))
                        for dst, wmat, src in trigs:
                            for c in range(4):
                                rhs = _ap3(src, 8 * v0 + c, 4, 2, 8, nvh)
                                nc.tensor.matmul(
                                    _ap3(dst, 512 * h, 256, 2, 1, nvh),
                                    lhsT=wmat[:, kb, c, :],
                                    rhs=rhs, start=(c == 0), stop=(c == 3))
                    # epilogue pieces (h, nh, v0p, nvp): single pass
                    # normally; split near the kernel tail (last sample's
                    # kb2/kb3) to shorten the final serial chain
                    t1 = tmps.tile([128, 1004], _f16, tag="t1", name=f"t1{s}_{kb}")
                    t2 = tmps.tile([128, 1004], _f16, tag="t2", name=f"t2{s}_{kb}")
                    pw = tmps.tile([128, 1004], _f16, tag="pw", name=f"pw{s}_{kb}")
                    if tail_kb:
                        pieces = ((0, 1, 0, 251), (1, 1, 0, 180),
                                  (1, 1, 180, 71))
                    else:
                        pieces = ((0, 2, 0, 251),)
                    for h_, nh, v0p, nvp in pieces:
                        co_p = 512 * h_ + v0p   # psum col offset
                        co_t = 502 * h_ + v0p   # packed-tile col offset
                        pcv = _ap4(pc, co_p, 512, nh, 256, 2, 1, nvp)
                        psv = _ap4(ps_, co_p, 512, nh, 256, 2, 1, nvp)
                        t1v = _ap4(t1, co_t, 502, nh, 251, 2, 1, nvp)
                        t2v = _ap4(t2, co_t, 502, nh, 251, 2, 1, nvp)
                        nc.scalar.activation(
                            out=t1v, in_=pcv,
                            func=mybir.ActivationFunctionType.Square,
                            bias=(bias_sb[:, 0:1] if kb == 0 else 0.0))
                        if not sin_dve:  # ACT square
                            nc.scalar.activation(
                                out=t2v, in_=psv,
                                func=mybir.ActivationFunctionType.Square)
                        else:  # DVE: copy PSUM->fp16, then 2x-mode self-mult
                            cp = tmps.tile([128, 1004], _f16, tag="cp",
                                           name=f"cp{s}_{kb}")
                            cpv = _ap4(cp, co_t, 502, nh, 251, 2, 1, nvp)
                            nc.vector.tensor_copy(out=cpv, in_=psv)
                            nc.vector.tensor_mul(
                                out=t2v, in0=cpv,
                                in1=_ap4(cp, co_t, 502, nh, 251, 2, 1, nvp))
                        nc.vector.tensor_add(out=_ap4(pw, co_t, 502, nh,
                                                      251, 2, 1, nvp),
                                             in0=t1v, in1=t2v)
                        # ln1p -> t-interleaved f16 out (t = 502h + 2v' + par)
                        obase = osb[:, 0:1]
                        oap = AP(obase.tensor,
                                 obase.offset + 1002 * kb + 502 * h_ + 2 * v0p,
                                 [list(obase.ap[0]), [502, nh], [1, 2],
                                  [2, nvp]])
                        pwv = _ap4(pw, co_t, 502, nh, 251, 2, 1, nvp)
                        nc.scalar.activation(
                            out=oap, in_=pwv,
                            func=mybir.ActivationFunctionType.Ln, bias=1.0)
                        # drain these bins/frames while the rest computes
                        tl = 502 * h_ + 2 * v0p
                        tr = min(tl + 2 * nvp, T) if nh == 1 else T
                        nc.sync.dma_start(
                            out=o_d[s, 128 * kb:128 * kb + 128, tl:tr],
                            in_=osb[:, 1002 * kb + tl:1002 * kb + tr],
                        )

                if s != SPC - 1:
                    emit_ny()

    nc.compile()
    return nc


def _host_prepare(x, lambd):
    """Build per-core slab inputs + folded DFT matrices (fp16)."""
    x = np.ascontiguousarray(x, dtype=np.float32)
    lam = float(np.asarray(lambd, dtype=np.float32))
    sq = np.sqrt(abs(lam)) if lam != 0 else 1.0

    xp = np.concatenate(
        [x[:, PAD:0:-1], x, x[:, L - 2: L - 2 - PAD: -1]], axis=1
    )  # [B, LP]
    nq = 128 * QS  # 320768 <= LP

    def slab(src, off):
        return np.ascontiguousarray(
            src[:, off:off + nq].reshape(B, QS, 128).transpose(0, 2, 1)
        ).astype(np.float16)

    xa = slab(xp, 1)   # xp[128q+p+1]
    xb = slab(xp, 65)  # xp[128q+p+65]

    # reversed-operand slabs, gathered in forward column order:
    # xrev*[p, 5v+c] = xp[base + 640v - 128c - p]
    vv = np.arange(NV)
    cc = np.arange(4)
    pp = np.arange(128)
    idx = (640 * vv[:, None, None] - 128 * cc[None, :, None]
           - pp[None, None, :])  # [NV, 4, 128]
    cols = (5 * vv[:, None] + cc[None, :]).ravel()  # 2004 used columns

    def revslab(base):
        iz = np.clip(base + idx, 0, LP - 1)
        vals = xp[:, iz]  # [B, NV, 4, 128]
        out = np.zeros((B, 128, QS), dtype=np.float16)
        out[:, :, cols] = vals.transpose(0, 3, 1, 2).reshape(B, 128, 4 * NV)
        return out

    xrevA = revslab(1023)  # xp[640v + 1023 - 128c - p]
    xrevB = revslab(1343)  # xp[640v + 1343 - 128c - p]
    x4 = np.ascontiguousarray(np.stack([xa, xb, xrevA, xrevB], axis=1))

    n = np.arange(1, 513, dtype=np.float64)  # contraction slots 1..512
    win = 0.5 * (1.0 - np.cos(2.0 * np.pi * n / N_FFT))
    k = np.arange(512, dtype=np.float64)
    ang = 2.0 * np.pi * np.outer(n, k) / N_FFT
    wc64 = sq * win[:, None] * np.cos(ang)
    ws64 = sq * win[:, None] * np.sin(ang)
    wc64[511, :] *= 0.5  # u[512] = 2*y[512]
    ws64[511, :] = 0.0
    wny64 = sq * win * np.cos(np.pi * n)
    wny64[511] = 0.5 * sq

    def to_pck(w):  # [512, 512k] -> [4kb, 128p, 4c, 128k], slot n=128c+p+1
        return np.ascontiguousarray(
            w.reshape(4, 128, 4, 128).transpose(2, 1, 0, 3)).astype(np.float16)

    wc = to_pck(wc64)
    ws = to_pck(ws64)
    wny = np.ascontiguousarray(
        wny64.reshape(4, 128).transpose(1, 0)).astype(np.float16)
    tmpl = np.zeros((128, 1), dtype=np.float32)
    tmpl[0, 0] = -512.0 * sq / L
    tmpl[1, 0] = 256.0 * sq / L
    return x4, wc, ws, wny, tmpl


def _in_maps(x4, wc, ws, wny, tmpl):
    maps = []
    for c in range(NCORES):
        sl = slice(c * SPC, (c + 1) * SPC)
        maps.append({
            "x4": np.ascontiguousarray(x4[sl]),
            "wc": wc, "ws": ws, "wny": wny, "tmpl": tmpl,
        })
    return maps


def kernel(x, lambd):
    from concourse.bass_utils import run_bass_kernel_spmd

    prep = _host_prepare(x, lambd)
    nc = _build_module()
    res = run_bass_kernel_spmd(nc, _in_maps(*prep), core_ids=list(range(NCORES)))
    out = np.concatenate([res.results[c]["o"] for c in range(NCORES)], axis=0)
    return out.astype(np.float32)


if __name__ == "__main__":
    rng = np.random.default_rng(0)
    x = rng.standard_normal((B, L), dtype=np.float32)
    out = kernel(x, np.float32(5.0))
    print(out.shape, out.dtype, out[0, :3, :3])



# revision 56
# speedup vs baseline: 1.2927x; 1.0109x over previous
"""Power-STFT kernel for Trainium2 (8 NeuronCores, data-parallel over batch).

Computes, for x [32, 320000] and scalar lambd:
    x <- x - mean(x, axis=1)
    power-STFT (n_fft=1024, hop=320, periodic Hann, center reflect pad)
    out = log1p(lambd * power)   -> [32, 513, 1001] fp32

Strategy per core (4 batch samples per core), v2 — folded DFT in fp16:
  - Window/trig symmetry: win(1024-n) = win(n), cos sym / sin antisym about
    n=512, so the windowed DFT reduces to a length-512 contraction over
      u_t[n] = y_t[n] + y_t[1024-n],  v_t[n] = y_t[n] - y_t[1024-n]
    (n = 1..512; u[512] = 2*y[512] absorbed with half weight; n=0 has
    win(0) = 0). This HALVES the tensor-engine work vs the direct 8-chunk
    form: 4 contraction chunks of 128 per trig.
  - All matmul inputs are fp16 (full PE rate, 1 row/cycle). Host ships four
    slab views of the reflect-padded signal so every fold operand is
    partition-aligned: xa/xb (forward, offsets +1/+65 so contraction slot
    (c,p) = sample 128c+p+1) and xrevA/xrevB (reversed: xp[321023-i] and
    xp[320959-i]) for even/odd frames. u/v are built by 4 DVE adds/subs per
    sample (fp16 2x mode) into a [128, v*8+par*4+c] layout whose matmul rhs
    slices are [128, stride-8 x 501].
  - Nyquist bin (512) rides data-stationary matmuls: lhsT = u frames chunk
    (M = 128 frames), rhs = folded (-1)^n window column (F=1) accumulated
    over 4 chunks into a [128 frames, 8 fgroup] PSUM tile; squared, PE-
    transposed via identity matmul, Ln'd, and DMA'd as output row 512.
  - Mean removal: periodic-Hann DFT of a constant is exactly [512, -256]
    at bins 0/1 (real), zero elsewhere. So demeaning == biasing bins 0/1
    of the cos part: bias vec = mu * [-512, 256, 0...] * sqrt(lambd),
    applied as the per-partition bias of the kb=0 cos ACT Square. mu comes
    from a GPSIMD column-reduce of xa + a 3-matmul partition-sum with
    edge fixups.
  - Epilogue per (kb): cos^2 on ACT (Square, PSUM strided view skipping
    bank-pad junk), sin^2 on DVE (scalar_tensor_tensor (ps*1)*ps), power
    add split DVE/GPSIMD, log1p via ACT Ln(power + 1) writing the t-
    interleaved fp32 out tile; one 2MB DMA per sample for bins 0..511.
  - sqrt(lambd) folded into the DFT matrices so power is pre-scaled.
"""

import sys

sys.path.insert(0, "/opt/trn_rl_repo")

import numpy as np

import concourse.bacc as bacc
import concourse.bass as bass
import concourse.mybir as mybir
import concourse.tile as tile
from concourse.ap import AP
from contextlib import ExitStack

N_FFT = 1024
HOP = 320
L = 320000
PAD = N_FFT // 2  # 512
LP = L + 2 * PAD  # 321024
B = 32
NCORES = 8
SPC = B // NCORES  # 4 samples per core
T = 1 + L // HOP  # 1001 frames
NV = 501  # even-frame count; odd frames use 500 + 1 junk col
QS = 2506  # slab columns
NBINS = 513

_f32 = mybir.dt.float32
_f16 = mybir.dt.float16


def _ap3(t, col_off, s1, n1, s2, n2):
    """[128, n1, n2] AP on tile t with free strides (s1, s2) from col_off."""
    base = t[:, 0:1]
    return AP(base.tensor, base.offset + col_off,
              [list(base.ap[0]), [s1, n1], [s2, n2]])


def _ap4(t, col_off, s1, n1, s2, n2, s3, n3):
    """[128, n1, n2, n3] AP on tile t with free strides (s1, s2, s3)."""
    base = t[:, 0:1]
    return AP(base.tensor, base.offset + col_off,
              [list(base.ap[0]), [s1, n1], [s2, n2], [s3, n3]])


def _build_module():
    nc = bacc.Bacc(None, target_bir_lowering=False, debug=False)

    x4_d = nc.dram_tensor("x4", [SPC, 4, 128, QS], _f16, kind="ExternalInput")
    wc_d = nc.dram_tensor("wc", [4, 128, 4, 128], _f16, kind="ExternalInput")
    ws_d = nc.dram_tensor("ws", [4, 128, 4, 128], _f16, kind="ExternalInput")
    wny_d = nc.dram_tensor("wny", [128, 4], _f16, kind="ExternalInput")
    tmpl_d = nc.dram_tensor("tmpl", [128, 1], _f32, kind="ExternalInput")
    o_d = nc.dram_tensor("o", [SPC, NBINS, T], _f16, kind="ExternalOutput")

    with tile.TileContext(nc) as tc:
        with ExitStack() as ctx:
            consts = ctx.enter_context(tc.tile_pool(name="consts", bufs=1))
            slabs = ctx.enter_context(tc.tile_pool(name="slabs", bufs=2))
            uvs = ctx.enter_context(tc.tile_pool(name="uvs", bufs=3))
            stats = ctx.enter_context(tc.tile_pool(name="stats", bufs=2))
            tmps = ctx.enter_context(tc.tile_pool(name="tmps", bufs=3))
            outs = ctx.enter_context(tc.tile_pool(name="outs", bufs=2))
            pmain = ctx.enter_context(tc.tile_pool(name="pmain", bufs=1, space="PSUM"))

            # v-range halves: (v0, nvh) — h0 sized so first-chunk DMAs
            # (cols < CSPLIT) cover all its fold reads
            HALVES = ((0, 251), (251, 250))
            CSPLIT = 1280

            # weights kb-major; par0's slabs (xa/xrA) first, kb0's cos
            # slice next so the first fold+matmul group starts earliest
            wc_sb = consts.tile([128, 4, 4, 128], _f16)  # [p, kb, c, k]
            ws_sb = consts.tile([128, 4, 4, 128], _f16)

            slab_tiles = []
            for s in range(SPC):
                tiles = []
                for i, tg in enumerate(("xa", "xb", "xrA", "xrB")):
                    t = slabs.tile([128, QS], _f16, tag=tg, name=f"{tg}{s}")
                    tiles.append(t)
                slab_tiles.append(tiles)
                if s == 0:
                    for i in (0, 2):
                        nc.sync.dma_start(out=tiles[i][:, 0:CSPLIT],
                                          in_=x4_d[s, i, :, 0:CSPLIT])
                    nc.sync.dma_start(out=wc_sb[:, 0], in_=wc_d[0])
                    for i in (1, 3):
                        nc.sync.dma_start(out=tiles[i][:, 0:CSPLIT],
                                          in_=x4_d[s, i, :, 0:CSPLIT])
                    nc.sync.dma_start(out=ws_sb[:, 0], in_=ws_d[0])
            for i, t in enumerate(slab_tiles[0]):
                nc.sync.dma_start(out=t[:, CSPLIT:QS],
                                  in_=x4_d[0, i, :, CSPLIT:QS])
            for kb in range(1, 4):
                nc.sync.dma_start(out=wc_sb[:, kb], in_=wc_d[kb])
                nc.sync.dma_start(out=ws_sb[:, kb], in_=ws_d[kb])
            wny_sb = consts.tile([128, 4], _f16)
            nc.sync.dma_start(out=wny_sb, in_=wny_d[:, :])
            tmplP = consts.tile([128, 1], _f32)
            nc.sync.dma_start(out=tmplP, in_=tmpl_d[:, :])

            for s in range(SPC):
                tiles = slab_tiles[s]
                if s != 0:
                    for i, t in enumerate(tiles):
                        nc.sync.dma_start(out=t[:, 0:CSPLIT],
                                          in_=x4_d[s, i, :, 0:CSPLIT])
                        nc.sync.dma_start(out=t[:, CSPLIT:QS],
                                          in_=x4_d[s, i, :, CSPLIT:QS])
                xa, xb, xrevA, xrevB = tiles

                # fold: u/v [128, 8v+4par+c], even par=0 from xa/xrevA,
                # odd par=1 from xb/xrevB (odd v=500 col is finite junk);
                # rev slabs are host-gathered in forward column order so all
                # strides are positive
                u = uvs.tile([128, 8 * NV], _f16, tag="u", name=f"u{s}")
                v = uvs.tile([128, 8 * NV], _f16, tag="v", name=f"v{s}")
                with tc.high_priority():
                    # folds gate the next sample's matmuls — schedule them
                    # ahead of epilogue work contending for DVE
                    for v0, nvh in HALVES:
                        for par, fwd, rev, off_f in ((0, xa, xrevA, 0),
                                                     (1, xb, xrevB, 2)):
                            in0 = _ap3(fwd, off_f + 5 * v0, 5, nvh, 1, 4)
                            in1 = _ap3(rev, 5 * v0, 5, nvh, 1, 4)
                            uo = _ap3(u, 4 * par + 8 * v0, 8, nvh, 1, 4)
                            vo = _ap3(v, 4 * par + 8 * v0, 8, nvh, 1, 4)
                            nc.vector.tensor_add(out=uo, in0=in0, in1=in1)
                            nc.vector.tensor_sub(out=vo, in0=in0, in1=in1)

                # mean chain: sum(xa[:, 4:2504]) == sum(x) up to two edge
                # samples (~1e-5 relative on mu — far below tolerance);
                # S = sS2[0]+sS2[1] broadcast to all partitions by a
                # stride-0 DMA, then bias = tmplcol * S on DVE (tmpl
                # carries /L and sqrt(lambd) scaling; rows 0/1 nonzero)
                sS2 = stats.tile([1, 2], _f32, tag="sS", name=f"sS{s}")
                nc.gpsimd.reduce_sum(out=sS2[:, 0:1], in_=xa[:, 4:CSPLIT],
                                     axis=mybir.AxisListType.XYZWC)
                nc.gpsimd.reduce_sum(out=sS2[:, 1:2], in_=xa[:, CSPLIT:2504],
                                     axis=mybir.AxisListType.XYZWC)
                Sb = stats.tile([1, 1], _f32, tag="Sb", name=f"Sb{s}")
                nc.vector.tensor_add(out=Sb, in0=sS2[:, 0:1], in1=sS2[:, 1:2])
                Sbrd = stats.tile([128, 1], _f32, tag="Sbrd", name=f"Sbrd{s}")
                nc.gpsimd.partition_broadcast(Sbrd[:, :], Sb[:, :])
                bias_sb = stats.tile([128, 1], _f32, tag="bias", name=f"bias{s}")
                nc.vector.tensor_mul(out=bias_sb, in0=tmplP[:, :], in1=Sbrd)

                def emit_ny(s=s, u=u):
                    # Nyquist bin: data-stationary chains, frames on out
                    # partitions; pny rides the ps rotation
                    misc = pmain.tile([128, 1024], _f32, tag="ps", bufs=2,
                                      name=f"misc{s}")
                    pny = misc[:, 0:8]
                    nc.vector.memset(misc[:, 7:8], 0.0)  # fg7 pad rows
                    ubase = u[:, 0:1]
                    for fg in range(8):
                        nv = 64 if fg < 7 else 53
                        m = 2 * nv
                        for c in range(4):
                            lhsT = AP(ubase.tensor,
                                      ubase.offset + 8 * 64 * fg + c,
                                      [list(ubase.ap[0]), [8, nv], [4, 2]])
                            nc.tensor.matmul(pny[0:m, fg:fg + 1], lhsT=lhsT,
                                             rhs=wny_sb[:, c:c + 1],
                                             start=(c == 0), stop=(c == 3))
                    syT = stats.tile([128, 8], _f16, tag="syT", name=f"syT{s}")
                    nc.scalar.activation(
                        out=syT, in_=pny,
                        func=mybir.ActivationFunctionType.Square)
                    nyrow = stats.tile([128, 8], _f16, tag="nyrow",
                                       name=f"ny{s}")
                    nc.scalar.activation(
                        out=nyrow, in_=syT,
                        func=mybir.ActivationFunctionType.Ln, bias=1.0)
                    # frame t = 128*fg + p: transposing gather DMAs
                    nc.sync.dma_start(
                        out=o_d[s, 512:513, 0:896].rearrange(
                            "one (fg p) -> p (one fg)", fg=7),
                        in_=nyrow[:, 0:7],
                    )
                    nc.sync.dma_start(
                        out=o_d[s, 512:513, 896:1001].rearrange(
                            "one p -> p one"),
                        in_=nyrow[0:105, 7:8])

                # pc/ps col layout: 512*h + 256*par + v' (one accumulation
                # group per (kb, h) = per bank, 2D (par, v') out AP);
                # t1/t2/cp/pw are quadrant-major packed: 502*h + 251*par + v'
                osb = outs.tile([128, 4 * 1002 + 2], _f16, tag="osb", name=f"osb{s}")
                if s == SPC - 1:
                    emit_ny()  # off the kernel tail: before kb0 (u is ready)
                for kb in range(4):
                    pc = pmain.tile([128, 1024], _f32, tag="pc", bufs=2, name=f"pc{s}_{kb}")
                    ps_ = pmain.tile([128, 1024], _f32, tag="ps", bufs=2, name=f"ps{s}_{kb}")
                    tail_kb = (s == SPC - 1 and kb == 3)
                    sin_dve = kb in (1, 3) or (s == SPC - 1 and kb == 2)
                    for h, (v0, nvh) in enumerate(HALVES):
                        # DVE-sin kbs: sin first so the DVE copy path starts
                        # while the cos matmuls still stream
                        trigs = (((ps_, ws_sb, v), (pc, wc_sb, u)) if sin_dve
                                 else ((pc, wc_sb, u), (ps_, ws_sb, v)))
                        # very first group: par-split subs (same bank, one
                        # start/stop pair) so par0 streams before xb/xrB land
                        if s == 0 and kb == 0 and h == 0:
                            par_subs = ((0, 1), (1, 1))
                        else:
                            par_subs = ((0, 2),)
                        for dst, wmat, src in trigs:
                            for si, (p0, np_) in enumerate(par_subs):
                                for c in range(4):
                                    rhs = _ap3(src, 8 * v0 + 4 * p0 + c,
                                               4, np_, 8, nvh)
                                    nc.tensor.matmul(
                                        _ap3(dst, 512 * h + 256 * p0,
                                             256, np_, 1, nvh),
                                        lhsT=wmat[:, kb, c, :],
                                        rhs=rhs,
                                        start=(c == 0 and si == 0),
                                        stop=(c == 3 and
                                              si == len(par_subs) - 1))
                    # epilogue pieces (h, nh, v0p, nvp): single pass
                    # normally; split near the kernel tail (last sample's
                    # kb2/kb3) to shorten the final serial chain
                    t1 = tmps.tile([128, 1004], _f16, tag="t1", name=f"t1{s}_{kb}")
                    t2 = tmps.tile([128, 1004], _f16, tag="t2", name=f"t2{s}_{kb}")
                    pw = tmps.tile([128, 1004], _f16, tag="pw", name=f"pw{s}_{kb}")
                    if tail_kb:
                        pieces = ((0, 1, 0, 251), (1, 1, 0, 180),
                                  (1, 1, 180, 71))
                    else:
                        pieces = ((0, 2, 0, 251),)
                    for h_, nh, v0p, nvp in pieces:
                        co_p = 512 * h_ + v0p   # psum col offset
                        co_t = 502 * h_ + v0p   # packed-tile col offset
                        pcv = _ap4(pc, co_p, 512, nh, 256, 2, 1, nvp)
                        psv = _ap4(ps_, co_p, 512, nh, 256, 2, 1, nvp)
                        t1v = _ap4(t1, co_t, 502, nh, 251, 2, 1, nvp)
                        t2v = _ap4(t2, co_t, 502, nh, 251, 2, 1, nvp)
                        nc.scalar.activation(
                            out=t1v, in_=pcv,
                            func=mybir.ActivationFunctionType.Square,
                            bias=(bias_sb[:, 0:1] if kb == 0 else 0.0))
                        if not sin_dve:  # ACT square
                            nc.scalar.activation(
                                out=t2v, in_=psv,
                                func=mybir.ActivationFunctionType.Square)
                        else:  # DVE: copy PSUM->fp16, then 2x-mode self-mult
                            cp = tmps.tile([128, 1004], _f16, tag="cp",
                                           name=f"cp{s}_{kb}")
                            cpv = _ap4(cp, co_t, 502, nh, 251, 2, 1, nvp)
                            nc.vector.tensor_copy(out=cpv, in_=psv)
                            nc.vector.tensor_mul(
                                out=t2v, in0=cpv,
                                in1=_ap4(cp, co_t, 502, nh, 251, 2, 1, nvp))
                        nc.vector.tensor_add(out=_ap4(pw, co_t, 502, nh,
                                                      251, 2, 1, nvp),
                                             in0=t1v, in1=t2v)
                        # ln1p -> t-interleaved f16 out (t = 502h + 2v' + par)
                        obase = osb[:, 0:1]
                        oap = AP(obase.tensor,
                                 obase.offset + 1002 * kb + 502 * h_ + 2 * v0p,
                                 [list(obase.ap[0]), [502, nh], [1, 2],
                                  [2, nvp]])
                        pwv = _ap4(pw, co_t, 502, nh, 251, 2, 1, nvp)
                        nc.scalar.activation(
                            out=oap, in_=pwv,
                            func=mybir.ActivationFunctionType.Ln, bias=1.0)
                        # drain these bins/frames while the rest computes
                        tl = 502 * h_ + 2 * v0p
                        tr = min(tl + 2 * nvp, T) if nh == 1 else T
                        nc.sync.dma_start(
                            out=o_d[s, 128 * kb:128 * kb + 128, tl:tr],
                            in_=osb[:, 1002 * kb + tl:1002 * kb + tr],
                        )

                if s != SPC - 1:
                    emit_ny()

    nc.compile()
    return nc


def _host_prepare(x, lambd):
    """Build per-core slab inputs + folded DFT matrices (fp16)."""
    x = np.ascontiguousarray(x, dtype=np.float32)
    lam = float(np.asarray(lambd, dtype=np.float32))
    sq = np.sqrt(abs(lam)) if lam != 0 else 1.0

    xp = np.concatenate(
        [x[:, PAD:0:-1], x, x[:, L - 2: L - 2 - PAD: -1]], axis=1
    )  # [B, LP]
    nq = 128 * QS  # 320768 <= LP

    def slab(src, off):
        return np.ascontiguousarray(
            src[:, off:off + nq].reshape(B, QS, 128).transpose(0, 2, 1)
        ).astype(np.float16)

    xa = slab(xp, 1)   # xp[128q+p+1]
    xb = slab(xp, 65)  # xp[128q+p+65]

    # reversed-operand slabs, gathered in forward column order:
    # xrev*[p, 5v+c] = xp[base + 640v - 128c - p]
    vv = np.arange(NV)
    cc = np.arange(4)
    pp = np.arange(128)
    idx = (640 * vv[:, None, None] - 128 * cc[None, :, None]
           - pp[None, None, :])  # [NV, 4, 128]
    cols = (5 * vv[:, None] + cc[None, :]).ravel()  # 2004 used columns

    def revslab(base):
        iz = np.clip(base + idx, 0, LP - 1)
        vals = xp[:, iz]  # [B, NV, 4, 128]
        out = np.zeros((B, 128, QS), dtype=np.float16)
        out[:, :, cols] = vals.transpose(0, 3, 1, 2).reshape(B, 128, 4 * NV)
        return out

    xrevA = revslab(1023)  # xp[640v + 1023 - 128c - p]
    xrevB = revslab(1343)  # xp[640v + 1343 - 128c - p]
    x4 = np.ascontiguousarray(np.stack([xa, xb, xrevA, xrevB], axis=1))

    n = np.arange(1, 513, dtype=np.float64)  # contraction slots 1..512
    win = 0.5 * (1.0 - np.cos(2.0 * np.pi * n / N_FFT))
    k = np.arange(512, dtype=np.float64)
    ang = 2.0 * np.pi * np.outer(n, k) / N_FFT
    wc64 = sq * win[:, None] * np.cos(ang)
    ws64 = sq * win[:, None] * np.sin(ang)
    wc64[511, :] *= 0.5  # u[512] = 2*y[512]
    ws64[511, :] = 0.0
    wny64 = sq * win * np.cos(np.pi * n)
    wny64[511] = 0.5 * sq

    def to_pck(w):  # [512, 512k] -> [4kb, 128p, 4c, 128k], slot n=128c+p+1
        return np.ascontiguousarray(
            w.reshape(4, 128, 4, 128).transpose(2, 1, 0, 3)).astype(np.float16)

    wc = to_pck(wc64)
    ws = to_pck(ws64)
    wny = np.ascontiguousarray(
        wny64.reshape(4, 128).transpose(1, 0)).astype(np.float16)
    tmpl = np.zeros((128, 1), dtype=np.float32)
    tmpl[0, 0] = -512.0 * sq / L
    tmpl[1, 0] = 256.0 * sq / L
    return x4, wc, ws, wny, tmpl


def _in_maps(x4, wc, ws, wny, tmpl):
    maps = []
    for c in range(NCORES):
        sl = slice(c * SPC, (c + 1) * SPC)
        maps.append({
            "x4": np.ascontiguousarray(x4[sl]),
            "wc": wc, "ws": ws, "wny": wny, "tmpl": tmpl,
        })
    return maps


def kernel(x, lambd):
    from concourse.bass_utils import run_bass_kernel_spmd

    prep = _host_prepare(x, lambd)
    nc = _build_module()
    res = run_bass_kernel_spmd(nc, _in_maps(*prep), core_ids=list(range(NCORES)))
    out = np.concatenate([res.results[c]["o"] for c in range(NCORES)], axis=0)
    return out.astype(np.float32)


if __name__ == "__main__":
    rng = np.random.default_rng(0)
    x = rng.standard_normal((B, L), dtype=np.float32)
    out = kernel(x, np.float32(5.0))
    print(out.shape, out.dtype, out[0, :3, :3])

